# revision 50
# baseline (speedup 1.0000x reference)
"""DCRNN layer (diffusion-conv GRU cell) as a Trainium2 Bass kernel.

Math: the reference computes, per dconv,
    out = sum_{k=1..K} A^k H0 W_{k-1} + bias
where A T = scatter_add(ew * T[src] -> dst) over the edge list. We rewrite
with Horner's rule:
    out = A(Y_1 + A(Y_2 + ... + A(Y_K)...)) + bias,   Y_k = H0 @ W_{k-1}
so every sparse hop runs on OUT_C-wide tensors (64) instead of 96-wide.

Sharding: nodes are padded to a multiple of 128*8 and split into 128-row
"windows"; core c owns a contiguous 1/8 of the windows.  Edges are sorted by
dst and routed to the core owning their dst window.  A hop gathers src rows
(f32, all B*OUT_C=256 columns at once) per 128-edge chunk, builds a one-hot
scatter matrix S[e, dst_local]=ew[e] on DVE, and accumulates S^T @ msgs into
PSUM on the PE.

Collective overlap: each core's rows are split into part A (SPLIT_W windows)
and part B.  Two separate AllGathers rebuild tfA / tfB between hops; AG-A
fires mid-loop (hidden under compute).  Each hop runs in two passes: pass 1
consumes only tfA (available early) for every window's A-half chunks and
stages acc+Y_j partials in DRAM; pass 2 consumes tfB, finishes each window,
and feeds the next AGs.  The next hop's pass 1 therefore overlaps this hop's
tail and both AG wires, leaving no exposed collective time in steady state.

The hop tensor chain must stay f32: pre-activation dconv outputs are ~60x the
final output scale, so T-chain quantization error is amplified ~60x (bf16
measured 0.23 rel err vs the 2e-2 gate; fp16 predicts ~0.029).
"""

import numpy as np
import ml_dtypes

import concourse.bacc as bacc
import concourse.mybir as mybir
import concourse.tile as tile
from concourse import bass_utils

NCORES = 8
WIN = 128
SPLIT_W = 10    # windows per core in AllGather part A (rest in part B)
F32 = mybir.dt.float32
HDT = mybir.dt.float32   # hop dtype: H storage / messages / one-hot matmuls
# NOTE: bf16/fp16 here fail accuracy: pre-activation dconv outputs are ~60x the
# final output scale, so T-chain quantization error is amplified ~60x (bf16
# measured 0.23 rel err vs the 2e-2 gate).
I16 = mybir.dt.int16


def _prep_edges(edge_index, edge_weight, n_pad, wpc):
    """Route edges to the core owning their dst window; within each window,
    slots are split [half-A chunks | half-B chunks] by the src node's half
    (the two split AllGathers write separate half tensors tfA/tfB).
    Gather idx values are rows within the half tensor: c*half_r + l%half_r.
    """
    src = edge_index[0].astype(np.int64)
    dst = edge_index[1].astype(np.int64)
    rows = wpc * WIN
    aw_r = SPLIT_W * WIN
    bw_r = rows - aw_r
    c_of, l_of = src // rows, src % rows
    halfid = (l_of >= aw_r).astype(np.int64)
    hrow = np.where(halfid == 0, c_of * aw_r + l_of,
                    c_of * bw_r + (l_of - aw_r))
    n_win = n_pad // WIN
    w_of = dst // WIN
    order = np.lexsort((hrow, halfid, w_of))
    sh, shrow = halfid[order], hrow[order]
    sdst, sew = dst[order], edge_weight[order].astype(np.float32)
    sw = w_of[order]

    cntA = np.bincount(sw[sh == 0], minlength=n_win)
    cntB = np.bincount(sw[sh == 1], minlength=n_win)
    chA = max(1, int(-(-cntA.max() // WIN)))
    chB = max(1, int(-(-cntB.max() // WIN)))
    nidx_w = (chA + chB) * WIN

    tot = n_win * nidx_w
    idx_all = np.zeros(tot, np.int64)
    dl_all = np.full(tot, -1.0, np.float32)
    ew_all = np.zeros(tot, np.float32)
    cnt_tot = cntA + cntB
    base = np.zeros(n_win, np.int64)
    np.cumsum(cnt_tot[:-1], out=base[1:])
    grp_start = np.where(sh == 0, base[sw], base[sw] + cntA[sw])
    rank = np.arange(len(sdst)) - grp_start
    gpos = sw * nidx_w + np.where(sh == 0, 0, chA * WIN) + rank
    idx_all[gpos] = shrow
    dl_all[gpos] = (sdst % WIN).astype(np.float32)
    ew_all[gpos] = sew
    # pad slots re-gather the last real row of their (window, half) block so
    # the wasted reads hit the same HBM page instead of hammering row 0
    for w in range(n_win):
        baseA = w * nidx_w
        a_end = baseA + int(cntA[w])
        if 0 < cntA[w] < chA * WIN:
            idx_all[a_end:baseA + chA * WIN] = idx_all[a_end - 1]
        baseB = baseA + chA * WIN
        b_end = baseB + int(cntB[w])
        if 0 < cntB[w] < chB * WIN:
            idx_all[b_end:baseB + chB * WIN] = idx_all[b_end - 1]

    per_core = []
    seg_len = wpc * nidx_w
    for c in range(NCORES):
        seg = slice(c * seg_len, (c + 1) * seg_len)
        idx = idx_all[seg]
        idx16 = np.tile(idx.astype(np.int16).reshape(-1, 16).T, (8, 1)).copy()
        dl = dl_all[seg].reshape(-1, 128).T.copy()
        ew = ew_all[seg].reshape(-1, 128).T.copy()
        per_core.append((idx16, dl, ew))
    # shared-program per-(local window, half) gather counts: max over cores
    maxA = cntA.reshape(NCORES, wpc).max(axis=0)
    maxB = cntB.reshape(NCORES, wpc).max(axis=0)
    return per_core, chA, chB, maxA, maxB


def _build_program(n_pad, wpc, chA, chB, maxA, maxB, b, in_c, out_c, k_hops, cols):
    rows = wpc * WIN
    ch = chA + chB
    nidx_w = ch * WIN
    totc = wpc * ch
    tot16 = wpc * nidx_w // 16
    koc = k_hops * out_c
    cmax = max(chA, chB)
    cseg = -(-cmax // -(-cmax // 9))  # balanced segments of <= 9 chunks

    nc = bacc.Bacc("TRN2", num_devices=NCORES, target_bir_lowering=False,
                   debug=False, num_swdge_queues=4,
                   dynamic_dma_scratch_size=32768)
    t_xT = nc.dram_tensor("xT", [in_c, b, rows], F32, kind="ExternalInput")
    t_hT = nc.dram_tensor("hT", [out_c, b, rows], F32, kind="ExternalInput")
    t_hrows = nc.dram_tensor("hrows", [rows, cols], F32, kind="ExternalInput")
    t_wx = nc.dram_tensor("wx", [in_c, koc], F32, kind="ExternalInput")
    t_wh = nc.dram_tensor("wh", [out_c, koc], F32, kind="ExternalInput")
    t_biasr = nc.dram_tensor("biasr", [128, cols], F32, kind="ExternalInput")
    t_iota = nc.dram_tensor("iota", [128, 128], F32, kind="ExternalInput")
    t_ident = nc.dram_tensor("ident", [128, 128], F32, kind="ExternalInput")
    t_idx = nc.dram_tensor("idx", [128, tot16], I16, kind="ExternalInput")
    t_dl = nc.dram_tensor("dstloc", [128, totc], F32, kind="ExternalInput")
    t_ew = nc.dram_tensor("ew", [128, totc], F32, kind="ExternalInput")
    t_out = nc.dram_tensor("out_rows", [rows, cols], F32, kind="ExternalOutput")

    with tile.TileContext(nc) as tc:
        with (tc.tile_pool(name="cons", bufs=1) as cons,
              tc.tile_pool(name="sbuf", bufs=4) as pool,
              tc.tile_pool(name="gbuf", bufs=6) as gbuf,
              tc.tile_pool(name="psum", bufs=3, space="PSUM") as psum,
              tc.tile_pool(name="psy", bufs=2, space="PSUM") as psy,
              tc.tile_pool(name="dram", bufs=3, space="DRAM") as dram):
            wx_t = cons.tile([in_c, koc], F32)
            nc.sync.dma_start(wx_t[:], t_wx[:])
            wh_t = cons.tile([out_c, koc], F32)
            nc.sync.dma_start(wh_t[:], t_wh[:])
            biasr_t = cons.tile([128, cols], F32)
            nc.sync.dma_start(biasr_t[:], t_biasr[:])
            iota_t = cons.tile([128, 128], F32)
            nc.sync.dma_start(iota_t[:], t_iota[:])
            ident_t = cons.tile([128, 128], F32)
            nc.sync.dma_start(ident_t[:], t_ident[:])
            idx_t = cons.tile([128, tot16], I16)
            nc.sync.dma_start(idx_t[:], t_idx[:])
            dl_t = cons.tile([128, totc], F32)
            nc.sync.dma_start(dl_t[:], t_dl[:])
            ew_t = cons.tile([128, totc], F32)
            nc.sync.dma_start(ew_t[:], t_ew[:])
            hbuf = cons.tile([128, wpc, cols], F32)
            nc.sync.dma_start(hbuf[:], t_hrows[:].rearrange("(w p) c -> p w c", p=128))
            zbuf = cons.tile([128, wpc, cols], F32)

            aw_r = SPLIT_W * WIN
            bw_r = rows - aw_r

            def ag_half(ag_h, tf_h):
                # AllGather of one half of each core's rows into its own
                # Shared half tensor (row = core*half_r + l % half_r).
                nc.gpsimd.collective_compute(
                    "AllGather", mybir.AluOpType.bypass,
                    replica_groups=[list(range(NCORES))],
                    ins=[ag_h.opt()], outs=[tf_h.opt()])

            def ag_slot(agA, agB, w):
                if w < SPLIT_W:
                    return agA, slice(w * WIN, (w + 1) * WIN)
                return agB, slice(w * WIN - aw_r, (w + 1) * WIN - aw_r)

            def y_window(second, w, agA, agB, ydram):
                # Y_k = H0 @ W_{k-1}; H0 = [x, h] or [x, z*h].  Writes Y_K
                # into ag_in and Y_{K-1}..Y_1 (f32) into ydram list.
                ystages = [pool.tile([128, cols], F32, tag=f"ys{j}",
                                      name=f"ys{j}")
                           for j in range(k_hops - 1)]
                ytop = pool.tile([128, cols], HDT, tag="ytop")
                for bi in range(b):
                    lx = pool.tile([in_c, 128], F32, tag="lx")
                    nc.sync.dma_start(
                        lx[:], t_xT[:, bi, w * WIN:(w + 1) * WIN])
                    lh = pool.tile([out_c, 128], F32, tag="lh")
                    if not second:
                        nc.sync.dma_start(
                            lh[:], t_hT[:, bi, w * WIN:(w + 1) * WIN])
                    else:
                        zh = pool.tile([128, out_c], F32, tag="zh")
                        csl = slice(bi * out_c, (bi + 1) * out_c)
                        nc.vector.tensor_mul(
                            zh[:], zbuf[:, w, csl], hbuf[:, w, csl])
                        tp = psy.tile([out_c, 128], F32, tag="tp")
                        nc.tensor.transpose(tp[:], zh[:], ident_t[:])
                        nc.vector.tensor_copy(lh[:], tp[:])
                    yp = psy.tile([128, koc], F32, tag="yp")
                    nc.tensor.matmul(yp[:], lhsT=lx[:], rhs=wx_t[:],
                                     start=True, stop=False)
                    nc.tensor.matmul(yp[:], lhsT=lh[:], rhs=wh_t[:],
                                     start=False, stop=True)
                    csl = slice(bi * out_c, (bi + 1) * out_c)
                    nc.vector.tensor_copy(
                        ytop[:, csl], yp[:, (k_hops - 1) * out_c:])
                    for j in range(k_hops - 1):
                        nc.vector.tensor_copy(
                            ystages[j][:, csl],
                            yp[:, j * out_c:(j + 1) * out_c])
                rsl = slice(w * WIN, (w + 1) * WIN)
                ag_t, asl = ag_slot(agA, agB, w)
                nc.sync.dma_start(ag_t[asl, :], ytop[:])
                for j in range(k_hops - 1):
                    nc.sync.dma_start(ydram[j][rsl, :], ystages[j][:])

            qctr = [0]

            def gather_half(tfh, w, c_lo, c_cnt, wcnt_max):
                # partial acc over one src-half's chunks of window w
                acc = psum.tile([128, cols], F32, tag="acc")
                for si, s0 in enumerate(range(0, c_cnt, cseg)):
                    ncs = min(cseg, c_cnt - s0)
                    c0 = c_lo + s0
                    # NOTE: num_idxs_reg < num_idxs hangs the runtime's sem
                    # accounting, so pads are gathered too (wcnt_max unused).
                    g = gbuf.tile([128, cseg, cols], HDT, tag="g")
                    off16 = w * (nidx_w // 16) + c0 * 8
                    qctr[0] += 1
                    nc.gpsimd.dma_gather(
                        out_ap=g[:, :ncs, :], in_ap=tfh[:],
                        idxs_ap=idx_t[:, off16:off16 + ncs * 8],
                        num_idxs=ncs * 128, num_idxs_reg=ncs * 128,
                        elem_size=cols, single_packet=False,
                        queue_num=qctr[0] % 4)
                    s_big = gbuf.tile([128, cseg * 128], HDT, tag="sbig")
                    s3 = s_big[:, :ncs * 128].rearrange(
                        "p (c j) -> p c j", c=ncs)
                    csl = slice(w * ch + c0, w * ch + c0 + ncs)
                    dl_exp = dl_t[:, csl] \
                        .rearrange("p (c o) -> p c o", o=1) \
                        .to_broadcast([128, ncs, 128])
                    ew_exp = ew_t[:, csl] \
                        .rearrange("p (c o) -> p c o", o=1) \
                        .to_broadcast([128, ncs, 128])
                    iota_exp = iota_t[:].rearrange("p (c j) -> p c j", c=1) \
                        .to_broadcast([128, ncs, 128])
                    nc.vector.tensor_tensor(out=s3, in0=iota_exp, in1=dl_exp,
                                            op=mybir.AluOpType.is_equal)
                    nc.vector.tensor_tensor(out=s3, in0=s3, in1=ew_exp,
                                            op=mybir.AluOpType.mult)
                    for ci in range(ncs):
                        nc.tensor.matmul(
                            acc[:], lhsT=s_big[:, ci * 128:(ci + 1) * 128],
                            rhs=g[:, ci, :], start=(s0 + ci == 0),
                            stop=(s0 + ci == c_cnt - 1))
                return acc

            def a_window(tfA, ydram_j, part, w, is_final):
                # pass 1: A-chunk partial + Y_j (or bias) -> part[w] in DRAM
                acc = gather_half(tfA, w, 0, chA, maxA[w])
                pw = pool.tile([128, cols], F32, tag="pw")
                if is_final:
                    nc.vector.tensor_add(pw[:], acc[:], biasr_t[:])
                else:
                    yj = pool.tile([128, cols], F32, tag="yj")
                    nc.sync.dma_start(yj[:], ydram_j[w * WIN:(w + 1) * WIN, :])
                    nc.vector.tensor_add(pw[:], acc[:], yj[:])
                nc.sync.dma_start(part[w * WIN:(w + 1) * WIN, :], pw[:])

            def mid_window_b(tfB, part, agA_n, agB_n, w):
                # pass 2: B-chunk partial + staged partial -> next hop input
                acc = gather_half(tfB, w, chA, chB, maxB[w])
                pl = pool.tile([128, cols], F32, tag="pl")
                nc.sync.dma_start(pl[:], part[w * WIN:(w + 1) * WIN, :])
                urow = pool.tile([128, cols], HDT, tag="urow")
                nc.vector.tensor_add(urow[:], acc[:], pl[:])
                ag_t, asl = ag_slot(agA_n, agB_n, w)
                nc.sync.dma_start(ag_t[asl, :], urow[:])

            def final_window_b(tfB, part, w, second):
                acc = gather_half(tfB, w, chA, chB, maxB[w])
                pl = pool.tile([128, cols], F32, tag="pl")
                nc.sync.dma_start(pl[:], part[w * WIN:(w + 1) * WIN, :])
                tmpf = pool.tile([128, cols], F32, tag="tmpf")
                nc.vector.tensor_add(tmpf[:], acc[:], pl[:])
                if not second:
                    nc.scalar.activation(
                        zbuf[:, w, :], tmpf[:],
                        mybir.ActivationFunctionType.Sigmoid)
                else:
                    htl = pool.tile([128, cols], F32, tag="htl")
                    nc.scalar.activation(
                        htl[:], tmpf[:], mybir.ActivationFunctionType.Tanh)
                    d = pool.tile([128, cols], F32, tag="d")
                    nc.vector.tensor_sub(d[:], hbuf[:, w, :], htl[:])
                    nc.vector.tensor_mul(d[:], zbuf[:, w, :], d[:])
                    outw = pool.tile([128, cols], F32, tag="outw")
                    nc.vector.tensor_add(outw[:], d[:], htl[:])
                    nc.sync.dma_start(
                        t_out[w * WIN:(w + 1) * WIN, :], outw[:])

            def new_chain():
                agA = dram.tile([aw_r, cols], HDT, tag="agA")
                agB = dram.tile([bw_r, cols], HDT, tag="agB")
                tfA = dram.tile([NCORES * aw_r, cols], HDT, tag="tfA",
                                addr_space="Shared")
                tfB = dram.tile([NCORES * bw_r, cols], HDT, tag="tfB",
                                addr_space="Shared")
                return agA, agB, tfA, tfB

            part_ctr = [0]

            def new_part():
                part_ctr[0] += 1
                return dram.tile([rows, cols], F32, tag="part",
                                 name=f"part{part_ctr[0]}")

            # AG for part A of the next-hop input fires a couple of windows
            # after its inputs are stored (in-order gpsimd queue).
            FIRE0 = SPLIT_W + 1

            # ---- dconv 1: Y stage ----
            ydram = [dram.tile([rows, cols], F32, tag=f"yd{j}", name=f"yd{j}")
                     for j in range(k_hops - 1)]
            agA, agB, tfA, tfB = new_chain()
            for w in range(wpc):
                y_window(False, w, agA, agB, ydram)
                if w == FIRE0:
                    ag_half(agA, tfA)
            ag_half(agB, tfB)

            def hop_phases(ydrams, second):
                nonlocal_tf = {}

                def run(tfA, tfB):
                    tfs = (tfA, tfB)
                    for j in range(k_hops - 2, -1, -1):
                        part = new_part()
                        for w in range(wpc):
                            a_window(tfs[0], ydrams[j], part, w, False)
                        agA_n, agB_n, tfA_n, tfB_n = new_chain()
                        for w in range(wpc):
                            mid_window_b(tfs[1], part, agA_n, agB_n, w)
                            if w == FIRE0:
                                ag_half(agA_n, tfA_n)
                        ag_half(agB_n, tfB_n)
                        tfs = (tfA_n, tfB_n)
                    return tfs

                return run

            # ---- dconv 1: mid hops ----
            tfA, tfB = hop_phases(ydram, False)(tfA, tfB)
            # ---- final hop of dconv 1 merged with y_stage of dconv 2 ----
            ydram2 = [dram.tile([rows, cols], F32, tag=f"yd{j}", name=f"y2d{j}")
                      for j in range(k_hops - 1)]
            part = new_part()
            for w in range(wpc):
                a_window(tfA, None, part, w, True)
            agA2, agB2, tfA2, tfB2 = new_chain()
            for w in range(wpc):
                final_window_b(tfB, part, w, second=False)
                y_window(True, w, agA2, agB2, ydram2)
                if w == FIRE0:
                    ag_half(agA2, tfA2)
            ag_half(agB2, tfB2)
            # ---- dconv 2: mid hops ----
            tfA, tfB = hop_phases(ydram2, True)(tfA2, tfB2)
            # ---- final hop of dconv 2 ----
            part = new_part()
            for w in range(wpc):
                a_window(tfA, None, part, w, True)
            for w in range(wpc):
                final_window_b(tfB, part, w, second=True)

    nc.compile()
    return nc


def kernel(x, h, weight, bias, edge_index, edge_weight):
    x = np.asarray(x, np.float32)
    h = np.asarray(h, np.float32)
    weight = np.asarray(weight, np.float32)
    bias = np.asarray(bias, np.float32)
    edge_index = np.asarray(edge_index)
    edge_weight = np.asarray(edge_weight, np.float32)

    b, n, in_c = x.shape
    out_c = h.shape[2]
    k_hops = weight.shape[0]
    cols = b * out_c
    assert (cols * 4) % 256 == 0, cols

    n_pad = -(-n // (WIN * NCORES)) * WIN * NCORES
    wpc = n_pad // WIN // NCORES
    rows = wpc * WIN

    per_core_edges, chA, chB, maxA, maxB = _prep_edges(edge_index, edge_weight, n_pad, wpc)
    nc = _build_program(n_pad, wpc, chA, chB, maxA, maxB, b, in_c, out_c, k_hops, cols)

    x_pad = np.zeros((b, n_pad, in_c), np.float32)
    x_pad[:, :n] = x
    h_pad = np.zeros((b, n_pad, out_c), np.float32)
    h_pad[:, :n] = h
    xT = np.ascontiguousarray(x_pad.transpose(2, 0, 1))       # [in_c, b, n_pad]
    hT = np.ascontiguousarray(h_pad.transpose(2, 0, 1))       # [out_c, b, n_pad]
    hrows = np.ascontiguousarray(
        h_pad.transpose(1, 0, 2).reshape(n_pad, cols))        # [n_pad, b*out_c]
    w_all = np.ascontiguousarray(
        weight.transpose(1, 0, 2).reshape(in_c + out_c, k_hops * out_c))
    biasr = np.tile(bias, (128, b)).astype(np.float32)
    iota = np.broadcast_to(np.arange(128, dtype=np.float32), (128, 128)).copy()
    ident = np.eye(128, dtype=np.float32)

    in_maps = []
    for c in range(NCORES):
        rsl = slice(c * rows, (c + 1) * rows)
        idx16, dl, ew = per_core_edges[c]
        in_maps.append({
            "xT": np.ascontiguousarray(xT[:, :, rsl]),
            "hT": np.ascontiguousarray(hT[:, :, rsl]),
            "hrows": np.ascontiguousarray(hrows[rsl]),
            "wx": np.ascontiguousarray(w_all[:in_c]),
            "wh": np.ascontiguousarray(w_all[in_c:]),
            "biasr": biasr, "iota": iota, "ident": ident,
            "idx": idx16, "dstloc": dl, "ew": ew,
        })

    res = bass_utils.run_bass_kernel_spmd(nc, in_maps, core_ids=list(range(NCORES)))
    kernel._last_results = res

    full = np.concatenate([res.results[c]["out_rows"] for c in range(NCORES)], axis=0)
    out = full[:n].reshape(n, b, out_c).transpose(1, 0, 2)
    return np.ascontiguousarray(out, dtype=np.float32)



# revision 51
# speedup vs baseline: 1.0015x; 1.0015x over previous
"""DCRNN layer (diffusion-conv GRU cell) as a Trainium2 Bass kernel.

Math: the reference computes, per dconv,
    out = sum_{k=1..K} A^k H0 W_{k-1} + bias
where A T = scatter_add(ew * T[src] -> dst) over the edge list. We rewrite
with Horner's rule:
    out = A(Y_1 + A(Y_2 + ... + A(Y_K)...)) + bias,   Y_k = H0 @ W_{k-1}
so every sparse hop runs on OUT_C-wide tensors (64) instead of 96-wide.

Sharding: nodes are padded to a multiple of 128*8 and split into 128-row
"windows"; core c owns a contiguous 1/8 of the windows.  Edges are sorted by
dst and routed to the core owning their dst window.  A hop gathers src rows
(f32, all B*OUT_C=256 columns at once) per 128-edge chunk, builds a one-hot
scatter matrix S[e, dst_local]=ew[e] on DVE, and accumulates S^T @ msgs into
PSUM on the PE.

Collective overlap: each core's rows are split into part A (SPLIT_W windows)
and part B.  Two separate AllGathers rebuild tfA / tfB between hops; AG-A
fires mid-loop (hidden under compute).  Each hop runs in two passes: pass 1
consumes only tfA (available early) for every window's A-half chunks and
stages acc+Y_j partials in DRAM; pass 2 consumes tfB, finishes each window,
and feeds the next AGs.  The next hop's pass 1 therefore overlaps this hop's
tail and both AG wires, leaving no exposed collective time in steady state.

The hop tensor chain must stay f32: pre-activation dconv outputs are ~60x the
final output scale, so T-chain quantization error is amplified ~60x (bf16
measured 0.23 rel err vs the 2e-2 gate; fp16 predicts ~0.029).
"""

import numpy as np
import ml_dtypes

import concourse.bacc as bacc
import concourse.mybir as mybir
import concourse.tile as tile
from concourse import bass_utils

NCORES = 8
WIN = 128
SPLIT_W = 10    # windows per core in AllGather part A (rest in part B)
F32 = mybir.dt.float32
HDT = mybir.dt.float32   # hop dtype: H storage / messages / one-hot matmuls
# NOTE: bf16/fp16 here fail accuracy: pre-activation dconv outputs are ~60x the
# final output scale, so T-chain quantization error is amplified ~60x (bf16
# measured 0.23 rel err vs the 2e-2 gate).
I16 = mybir.dt.int16


def _prep_edges(edge_index, edge_weight, n_pad, wpc):
    """Route edges to the core owning their dst window; within each window,
    slots are split [half-A chunks | half-B chunks] by the src node's half
    (the two split AllGathers write separate half tensors tfA/tfB).
    Gather idx values are rows within the half tensor: c*half_r + l%half_r.
    """
    src = edge_index[0].astype(np.int64)
    dst = edge_index[1].astype(np.int64)
    rows = wpc * WIN
    aw_r = SPLIT_W * WIN
    bw_r = rows - aw_r
    c_of, l_of = src // rows, src % rows
    halfid = (l_of >= aw_r).astype(np.int64)
    hrow = np.where(halfid == 0, c_of * aw_r + l_of,
                    c_of * bw_r + (l_of - aw_r))
    n_win = n_pad // WIN
    w_of = dst // WIN
    order = np.lexsort((hrow, halfid, w_of))
    sh, shrow = halfid[order], hrow[order]
    sdst, sew = dst[order], edge_weight[order].astype(np.float32)
    sw = w_of[order]

    cntA = np.bincount(sw[sh == 0], minlength=n_win)
    cntB = np.bincount(sw[sh == 1], minlength=n_win)
    chA = max(1, int(-(-cntA.max() // WIN)))
    chB = max(1, int(-(-cntB.max() // WIN)))
    nidx_w = (chA + chB) * WIN

    tot = n_win * nidx_w
    idx_all = np.zeros(tot, np.int64)
    dl_all = np.full(tot, -1.0, np.float32)
    ew_all = np.zeros(tot, np.float32)
    cnt_tot = cntA + cntB
    base = np.zeros(n_win, np.int64)
    np.cumsum(cnt_tot[:-1], out=base[1:])
    grp_start = np.where(sh == 0, base[sw], base[sw] + cntA[sw])
    rank = np.arange(len(sdst)) - grp_start
    gpos = sw * nidx_w + np.where(sh == 0, 0, chA * WIN) + rank
    idx_all[gpos] = shrow
    dl_all[gpos] = (sdst % WIN).astype(np.float32)
    ew_all[gpos] = sew
    # pad slots re-gather the last real row of their (window, half) block so
    # the wasted reads hit the same HBM page instead of hammering row 0
    for w in range(n_win):
        baseA = w * nidx_w
        a_end = baseA + int(cntA[w])
        if 0 < cntA[w] < chA * WIN:
            idx_all[a_end:baseA + chA * WIN] = idx_all[a_end - 1]
        baseB = baseA + chA * WIN
        b_end = baseB + int(cntB[w])
        if 0 < cntB[w] < chB * WIN:
            idx_all[b_end:baseB + chB * WIN] = idx_all[b_end - 1]

    per_core = []
    seg_len = wpc * nidx_w
    for c in range(NCORES):
        seg = slice(c * seg_len, (c + 1) * seg_len)
        idx = idx_all[seg]
        idx16 = np.tile(idx.astype(np.int16).reshape(-1, 16).T, (8, 1)).copy()
        dl = dl_all[seg].reshape(-1, 128).T.copy()
        ew = ew_all[seg].reshape(-1, 128).T.copy()
        per_core.append((idx16, dl, ew))
    # shared-program per-(local window, half) gather counts: max over cores
    maxA = cntA.reshape(NCORES, wpc).max(axis=0)
    maxB = cntB.reshape(NCORES, wpc).max(axis=0)
    return per_core, chA, chB, maxA, maxB


def _build_program(n_pad, wpc, chA, chB, maxA, maxB, b, in_c, out_c, k_hops, cols):
    rows = wpc * WIN
    ch = chA + chB
    nidx_w = ch * WIN
    totc = wpc * ch
    tot16 = wpc * nidx_w // 16
    koc = k_hops * out_c
    cmax = max(chA, chB)
    cseg = -(-cmax // -(-cmax // 9))  # balanced segments of <= 9 chunks

    nc = bacc.Bacc("TRN2", num_devices=NCORES, target_bir_lowering=False,
                   debug=False, num_swdge_queues=4,
                   dynamic_dma_scratch_size=32768)
    t_xT = nc.dram_tensor("xT", [in_c, b, rows], F32, kind="ExternalInput")
    t_hT = nc.dram_tensor("hT", [out_c, b, rows], F32, kind="ExternalInput")
    t_hrows = nc.dram_tensor("hrows", [rows, cols], F32, kind="ExternalInput")
    t_wx = nc.dram_tensor("wx", [in_c, koc], F32, kind="ExternalInput")
    t_wh = nc.dram_tensor("wh", [out_c, koc], F32, kind="ExternalInput")
    t_biasr = nc.dram_tensor("biasr", [128, cols], F32, kind="ExternalInput")
    t_iota = nc.dram_tensor("iota", [128, 128], F32, kind="ExternalInput")
    t_ident = nc.dram_tensor("ident", [128, 128], F32, kind="ExternalInput")
    t_idx = nc.dram_tensor("idx", [128, tot16], I16, kind="ExternalInput")
    t_dl = nc.dram_tensor("dstloc", [128, totc], F32, kind="ExternalInput")
    t_ew = nc.dram_tensor("ew", [128, totc], F32, kind="ExternalInput")
    t_out = nc.dram_tensor("out_rows", [rows, cols], F32, kind="ExternalOutput")

    with tile.TileContext(nc) as tc:
        with (tc.tile_pool(name="cons", bufs=1) as cons,
              tc.tile_pool(name="sbuf", bufs=3) as pool,
              tc.tile_pool(name="gbuf", bufs=6) as gbuf,
              tc.tile_pool(name="psum", bufs=3, space="PSUM") as psum,
              tc.tile_pool(name="psy", bufs=2, space="PSUM") as psy,
              tc.tile_pool(name="dram", bufs=3, space="DRAM") as dram):
            wx_t = cons.tile([in_c, koc], F32)
            nc.sync.dma_start(wx_t[:], t_wx[:])
            wh_t = cons.tile([out_c, koc], F32)
            nc.sync.dma_start(wh_t[:], t_wh[:])
            biasr_t = cons.tile([128, cols], F32)
            nc.sync.dma_start(biasr_t[:], t_biasr[:])
            iota_t = cons.tile([128, 128], F32)
            nc.sync.dma_start(iota_t[:], t_iota[:])
            ident_t = cons.tile([128, 128], F32)
            nc.sync.dma_start(ident_t[:], t_ident[:])
            idx_t = cons.tile([128, tot16], I16)
            nc.sync.dma_start(idx_t[:], t_idx[:])
            dl_t = cons.tile([128, totc], F32)
            nc.sync.dma_start(dl_t[:], t_dl[:])
            ew_t = cons.tile([128, totc], F32)
            nc.sync.dma_start(ew_t[:], t_ew[:])
            hbuf = cons.tile([128, wpc, cols], F32)
            nc.sync.dma_start(hbuf[:], t_hrows[:].rearrange("(w p) c -> p w c", p=128))
            zbuf = cons.tile([128, wpc, cols], F32)

            aw_r = SPLIT_W * WIN
            bw_r = rows - aw_r

            def ag_half(ag_h, tf_h):
                # AllGather of one half of each core's rows into its own
                # Shared half tensor (row = core*half_r + l % half_r).
                nc.gpsimd.collective_compute(
                    "AllGather", mybir.AluOpType.bypass,
                    replica_groups=[list(range(NCORES))],
                    ins=[ag_h.opt()], outs=[tf_h.opt()])

            def ag_slot(agA, agB, w):
                if w < SPLIT_W:
                    return agA, slice(w * WIN, (w + 1) * WIN)
                return agB, slice(w * WIN - aw_r, (w + 1) * WIN - aw_r)

            def y_window(second, w, agA, agB, ydram):
                # Y_k = H0 @ W_{k-1}; H0 = [x, h] or [x, z*h].  Writes Y_K
                # into ag_in and Y_{K-1}..Y_1 (f32) into ydram list.
                ystages = [pool.tile([128, cols], F32, tag=f"ys{j}",
                                      name=f"ys{j}")
                           for j in range(k_hops - 1)]
                ytop = pool.tile([128, cols], HDT, tag="ytop")
                for bi in range(b):
                    lx = pool.tile([in_c, 128], F32, tag="lx")
                    nc.sync.dma_start(
                        lx[:], t_xT[:, bi, w * WIN:(w + 1) * WIN])
                    lh = pool.tile([out_c, 128], F32, tag="lh")
                    if not second:
                        nc.sync.dma_start(
                            lh[:], t_hT[:, bi, w * WIN:(w + 1) * WIN])
                    else:
                        zh = pool.tile([128, out_c], F32, tag="zh")
                        csl = slice(bi * out_c, (bi + 1) * out_c)
                        nc.vector.tensor_mul(
                            zh[:], zbuf[:, w, csl], hbuf[:, w, csl])
                        tp = psy.tile([out_c, 128], F32, tag="tp")
                        nc.tensor.transpose(tp[:], zh[:], ident_t[:])
                        nc.vector.tensor_copy(lh[:], tp[:])
                    yp = psy.tile([128, koc], F32, tag="yp")
                    nc.tensor.matmul(yp[:], lhsT=lx[:], rhs=wx_t[:],
                                     start=True, stop=False)
                    nc.tensor.matmul(yp[:], lhsT=lh[:], rhs=wh_t[:],
                                     start=False, stop=True)
                    csl = slice(bi * out_c, (bi + 1) * out_c)
                    nc.vector.tensor_copy(
                        ytop[:, csl], yp[:, (k_hops - 1) * out_c:])
                    for j in range(k_hops - 1):
                        nc.vector.tensor_copy(
                            ystages[j][:, csl],
                            yp[:, j * out_c:(j + 1) * out_c])
                rsl = slice(w * WIN, (w + 1) * WIN)
                ag_t, asl = ag_slot(agA, agB, w)
                nc.sync.dma_start(ag_t[asl, :], ytop[:])
                for j in range(k_hops - 1):
                    nc.sync.dma_start(ydram[j][rsl, :], ystages[j][:])

            qctr = [0]

            def gather_half(tfh, w, c_lo, c_cnt, wcnt_max):
                # partial acc over one src-half's chunks of window w
                acc = psum.tile([128, cols], F32, tag="acc")
                for si, s0 in enumerate(range(0, c_cnt, cseg)):
                    ncs = min(cseg, c_cnt - s0)
                    c0 = c_lo + s0
                    # NOTE: num_idxs_reg < num_idxs hangs the runtime's sem
                    # accounting, so pads are gathered too (wcnt_max unused).
                    g = gbuf.tile([128, cseg, cols], HDT, tag="g")
                    off16 = w * (nidx_w // 16) + c0 * 8
                    qctr[0] += 1
                    nc.gpsimd.dma_gather(
                        out_ap=g[:, :ncs, :], in_ap=tfh[:],
                        idxs_ap=idx_t[:, off16:off16 + ncs * 8],
                        num_idxs=ncs * 128, num_idxs_reg=ncs * 128,
                        elem_size=cols, single_packet=False,
                        queue_num=qctr[0] % 4)
                    s_big = gbuf.tile([128, cseg * 128], HDT, tag="sbig")
                    s3 = s_big[:, :ncs * 128].rearrange(
                        "p (c j) -> p c j", c=ncs)
                    csl = slice(w * ch + c0, w * ch + c0 + ncs)
                    dl_exp = dl_t[:, csl] \
                        .rearrange("p (c o) -> p c o", o=1) \
                        .to_broadcast([128, ncs, 128])
                    ew_exp = ew_t[:, csl] \
                        .rearrange("p (c o) -> p c o", o=1) \
                        .to_broadcast([128, ncs, 128])
                    iota_exp = iota_t[:].rearrange("p (c j) -> p c j", c=1) \
                        .to_broadcast([128, ncs, 128])
                    nc.vector.tensor_tensor(out=s3, in0=iota_exp, in1=dl_exp,
                                            op=mybir.AluOpType.is_equal)
                    nc.vector.tensor_tensor(out=s3, in0=s3, in1=ew_exp,
                                            op=mybir.AluOpType.mult)
                    for ci in range(ncs):
                        nc.tensor.matmul(
                            acc[:], lhsT=s_big[:, ci * 128:(ci + 1) * 128],
                            rhs=g[:, ci, :], start=(s0 + ci == 0),
                            stop=(s0 + ci == c_cnt - 1))
                return acc

            def a_window(tfA, ydram_j, part, w, is_final):
                # pass 1: A-chunk partial + Y_j (or bias) -> part[w] in DRAM
                acc = gather_half(tfA, w, 0, chA, maxA[w])
                pw = pool.tile([128, cols], F32, tag="pw")
                if is_final:
                    nc.vector.tensor_add(pw[:], acc[:], biasr_t[:])
                else:
                    yj = pool.tile([128, cols], F32, tag="yj")
                    nc.sync.dma_start(yj[:], ydram_j[w * WIN:(w + 1) * WIN, :])
                    nc.vector.tensor_add(pw[:], acc[:], yj[:])
                nc.sync.dma_start(part[w * WIN:(w + 1) * WIN, :], pw[:])

            def mid_window_b(tfB, part, agA_n, agB_n, w):
                # pass 2: B-chunk partial + staged partial -> next hop input
                acc = gather_half(tfB, w, chA, chB, maxB[w])
                pl = pool.tile([128, cols], F32, tag="pl")
                nc.sync.dma_start(pl[:], part[w * WIN:(w + 1) * WIN, :])
                urow = pool.tile([128, cols], HDT, tag="urow")
                nc.vector.tensor_add(urow[:], acc[:], pl[:])
                ag_t, asl = ag_slot(agA_n, agB_n, w)
                nc.sync.dma_start(ag_t[asl, :], urow[:])

            def final_window_b(tfB, part, w, second):
                acc = gather_half(tfB, w, chA, chB, maxB[w])
                pl = pool.tile([128, cols], F32, tag="pl")
                nc.sync.dma_start(pl[:], part[w * WIN:(w + 1) * WIN, :])
                tmpf = pool.tile([128, cols], F32, tag="tmpf")
                nc.vector.tensor_add(tmpf[:], acc[:], pl[:])
                if not second:
                    nc.scalar.activation(
                        zbuf[:, w, :], tmpf[:],
                        mybir.ActivationFunctionType.Sigmoid)
                else:
                    htl = pool.tile([128, cols], F32, tag="htl")
                    nc.scalar.activation(
                        htl[:], tmpf[:], mybir.ActivationFunctionType.Tanh)
                    d = pool.tile([128, cols], F32, tag="d")
                    nc.vector.tensor_sub(d[:], hbuf[:, w, :], htl[:])
                    nc.vector.tensor_mul(d[:], zbuf[:, w, :], d[:])
                    outw = pool.tile([128, cols], F32, tag="outw")
                    nc.vector.tensor_add(outw[:], d[:], htl[:])
                    nc.sync.dma_start(
                        t_out[w * WIN:(w + 1) * WIN, :], outw[:])

            def new_chain():
                agA = dram.tile([aw_r, cols], HDT, tag="agA")
                agB = dram.tile([bw_r, cols], HDT, tag="agB")
                tfA = dram.tile([NCORES * aw_r, cols], HDT, tag="tfA",
                                addr_space="Shared")
                tfB = dram.tile([NCORES * bw_r, cols], HDT, tag="tfB",
                                addr_space="Shared")
                return agA, agB, tfA, tfB

            part_ctr = [0]

            def new_part():
                part_ctr[0] += 1
                return dram.tile([rows, cols], F32, tag="part",
                                 name=f"part{part_ctr[0]}")

            # AG for part A of the next-hop input fires a couple of windows
            # after its inputs are stored (in-order gpsimd queue).
            FIRE0 = SPLIT_W + 1

            # ---- dconv 1: Y stage ----
            ydram = [dram.tile([rows, cols], F32, tag=f"yd{j}", name=f"yd{j}")
                     for j in range(k_hops - 1)]
            agA, agB, tfA, tfB = new_chain()
            for w in range(wpc):
                y_window(False, w, agA, agB, ydram)
                if w == FIRE0:
                    ag_half(agA, tfA)
            ag_half(agB, tfB)

            def hop_phases(ydrams, second):
                nonlocal_tf = {}

                def run(tfA, tfB):
                    tfs = (tfA, tfB)
                    for j in range(k_hops - 2, -1, -1):
                        part = new_part()
                        for w in range(wpc):
                            a_window(tfs[0], ydrams[j], part, w, False)
                        agA_n, agB_n, tfA_n, tfB_n = new_chain()
                        for w in range(wpc):
                            mid_window_b(tfs[1], part, agA_n, agB_n, w)
                            if w == FIRE0:
                                ag_half(agA_n, tfA_n)
                        ag_half(agB_n, tfB_n)
                        tfs = (tfA_n, tfB_n)
                    return tfs

                return run

            # ---- dconv 1: mid hops ----
            tfA, tfB = hop_phases(ydram, False)(tfA, tfB)
            # ---- final hop of dconv 1 merged with y_stage of dconv 2 ----
            ydram2 = [dram.tile([rows, cols], F32, tag=f"yd{j}", name=f"y2d{j}")
                      for j in range(k_hops - 1)]
            part = new_part()
            for w in range(wpc):
                a_window(tfA, None, part, w, True)
            agA2, agB2, tfA2, tfB2 = new_chain()
            for w in range(wpc):
                final_window_b(tfB, part, w, second=False)
                y_window(True, w, agA2, agB2, ydram2)
                if w == FIRE0:
                    ag_half(agA2, tfA2)
            ag_half(agB2, tfB2)
            # ---- dconv 2: mid hops ----
            tfA, tfB = hop_phases(ydram2, True)(tfA2, tfB2)
            # ---- final hop of dconv 2 ----
            part = new_part()
            for w in range(wpc):
                a_window(tfA, None, part, w, True)
            for w in range(wpc):
                final_window_b(tfB, part, w, second=True)

    nc.compile()
    return nc


def kernel(x, h, weight, bias, edge_index, edge_weight):
    x = np.asarray(x, np.float32)
    h = np.asarray(h, np.float32)
    weight = np.asarray(weight, np.float32)
    bias = np.asarray(bias, np.float32)
    edge_index = np.asarray(edge_index)
    edge_weight = np.asarray(edge_weight, np.float32)

    b, n, in_c = x.shape
    out_c = h.shape[2]
    k_hops = weight.shape[0]
    cols = b * out_c
    assert (cols * 4) % 256 == 0, cols

    n_pad = -(-n // (WIN * NCORES)) * WIN * NCORES
    wpc = n_pad // WIN // NCORES
    rows = wpc * WIN

    per_core_edges, chA, chB, maxA, maxB = _prep_edges(edge_index, edge_weight, n_pad, wpc)
    nc = _build_program(n_pad, wpc, chA, chB, maxA, maxB, b, in_c, out_c, k_hops, cols)

    x_pad = np.zeros((b, n_pad, in_c), np.float32)
    x_pad[:, :n] = x
    h_pad = np.zeros((b, n_pad, out_c), np.float32)
    h_pad[:, :n] = h
    xT = np.ascontiguousarray(x_pad.transpose(2, 0, 1))       # [in_c, b, n_pad]
    hT = np.ascontiguousarray(h_pad.transpose(2, 0, 1))       # [out_c, b, n_pad]
    hrows = np.ascontiguousarray(
        h_pad.transpose(1, 0, 2).reshape(n_pad, cols))        # [n_pad, b*out_c]
    w_all = np.ascontiguousarray(
        weight.transpose(1, 0, 2).reshape(in_c + out_c, k_hops * out_c))
    biasr = np.tile(bias, (128, b)).astype(np.float32)
    iota = np.broadcast_to(np.arange(128, dtype=np.float32), (128, 128)).copy()
    ident = np.eye(128, dtype=np.float32)

    in_maps = []
    for c in range(NCORES):
        rsl = slice(c * rows, (c + 1) * rows)
        idx16, dl, ew = per_core_edges[c]
        in_maps.append({
            "xT": np.ascontiguousarray(xT[:, :, rsl]),
            "hT": np.ascontiguousarray(hT[:, :, rsl]),
            "hrows": np.ascontiguousarray(hrows[rsl]),
            "wx": np.ascontiguousarray(w_all[:in_c]),
            "wh": np.ascontiguousarray(w_all[in_c:]),
            "biasr": biasr, "iota": iota, "ident": ident,
            "idx": idx16, "dstloc": dl, "ew": ew,
        })

    res = bass_utils.run_bass_kernel_spmd(nc, in_maps, core_ids=list(range(NCORES)))
    kernel._last_results = res

    full = np.concatenate([res.results[c]["out_rows"] for c in range(NCORES)], axis=0)
    out = full[:n].reshape(n, b, out_c).transpose(1, 0, 2)
    return np.ascontiguousarray(out, dtype=np.float32)



# revision 52
# speedup vs baseline: 1.0219x; 1.0203x over previous
"""DCRNN layer (diffusion-conv GRU cell) as a Trainium2 Bass kernel.

Math: the reference computes, per dconv,
    out = sum_{k=1..K} A^k H0 W_{k-1} + bias
where A T = scatter_add(ew * T[src] -> dst) over the edge list. We rewrite
with Horner's rule:
    out = A(Y_1 + A(Y_2 + ... + A(Y_K)...)) + bias,   Y_k = H0 @ W_{k-1}
so every sparse hop runs on OUT_C-wide tensors (64) instead of 96-wide.

Sharding: nodes are padded to a multiple of 128*8 and split into 128-row
"windows"; core c owns a contiguous 1/8 of the windows.  Edges are sorted by
dst and routed to the core owning their dst window.  A hop gathers src rows
(f32, all B*OUT_C=256 columns at once) per 128-edge chunk, builds a one-hot
scatter matrix S[e, dst_local]=ew[e] on DVE, and accumulates S^T @ msgs into
PSUM on the PE.

Collective overlap: each core's rows are split into part A (SPLIT_W windows)
and part B.  Two separate AllGathers rebuild tfA / tfB between hops; AG-A
fires mid-loop (hidden under compute).  Each hop runs in two passes: pass 1
consumes only tfA (available early) for every window's A-half chunks and
stages acc+Y_j partials in DRAM; pass 2 consumes tfB, finishes each window,
and feeds the next AGs.  The next hop's pass 1 therefore overlaps this hop's
tail and both AG wires, leaving no exposed collective time in steady state.

The hop tensor chain must stay f32: pre-activation dconv outputs are ~60x the
final output scale, so T-chain quantization error is amplified ~60x (bf16
measured 0.23 rel err vs the 2e-2 gate; fp16 predicts ~0.029).
"""

import numpy as np
import ml_dtypes

import concourse.bacc as bacc
import concourse.mybir as mybir
import concourse.tile as tile
from concourse import bass_utils

NCORES = 8
WIN = 128
SPLIT_W = 10    # windows per core in AllGather part A (rest in part B)
F32 = mybir.dt.float32
HDT = mybir.dt.float32   # hop dtype: H storage / messages / one-hot matmuls
# NOTE: bf16/fp16 here fail accuracy: pre-activation dconv outputs are ~60x the
# final output scale, so T-chain quantization error is amplified ~60x (bf16
# measured 0.23 rel err vs the 2e-2 gate).
I16 = mybir.dt.int16


def _prep_edges(edge_index, edge_weight, n_pad, wpc):
    """Route edges to the core owning their dst window; within each window,
    slots are split [half-A chunks | half-B chunks] by the src node's half
    (the two split AllGathers write separate half tensors tfA/tfB).
    Gather idx values are rows within the half tensor: c*half_r + l%half_r.
    """
    src = edge_index[0].astype(np.int64)
    dst = edge_index[1].astype(np.int64)
    rows = wpc * WIN
    aw_r = SPLIT_W * WIN
    bw_r = rows - aw_r
    c_of, l_of = src // rows, src % rows
    halfid = (l_of >= aw_r).astype(np.int64)
    hrow = np.where(halfid == 0, c_of * aw_r + l_of,
                    c_of * bw_r + (l_of - aw_r))
    n_win = n_pad // WIN
    w_of = dst // WIN
    order = np.lexsort((hrow, halfid, w_of))
    sh, shrow = halfid[order], hrow[order]
    sdst, sew = dst[order], edge_weight[order].astype(np.float32)
    sw = w_of[order]

    cntA = np.bincount(sw[sh == 0], minlength=n_win)
    cntB = np.bincount(sw[sh == 1], minlength=n_win)
    chA = max(1, int(-(-cntA.max() // WIN)))
    chB = max(1, int(-(-cntB.max() // WIN)))
    nidx_w = (chA + chB) * WIN

    tot = n_win * nidx_w
    idx_all = np.zeros(tot, np.int64)
    dl_all = np.full(tot, -1.0, np.float32)
    ew_all = np.zeros(tot, np.float32)
    cnt_tot = cntA + cntB
    base = np.zeros(n_win, np.int64)
    np.cumsum(cnt_tot[:-1], out=base[1:])
    grp_start = np.where(sh == 0, base[sw], base[sw] + cntA[sw])
    rank = np.arange(len(sdst)) - grp_start
    gpos = sw * nidx_w + np.where(sh == 0, 0, chA * WIN) + rank
    idx_all[gpos] = shrow
    dl_all[gpos] = (sdst % WIN).astype(np.float32)
    ew_all[gpos] = sew
    # pad slots re-gather the last real row of their (window, half) block so
    # the wasted reads hit the same HBM page instead of hammering row 0
    for w in range(n_win):
        baseA = w * nidx_w
        a_end = baseA + int(cntA[w])
        if 0 < cntA[w] < chA * WIN:
            idx_all[a_end:baseA + chA * WIN] = idx_all[a_end - 1]
        baseB = baseA + chA * WIN
        b_end = baseB + int(cntB[w])
        if 0 < cntB[w] < chB * WIN:
            idx_all[b_end:baseB + chB * WIN] = idx_all[b_end - 1]

    per_core = []
    seg_len = wpc * nidx_w
    for c in range(NCORES):
        seg = slice(c * seg_len, (c + 1) * seg_len)
        idx = idx_all[seg]
        idx16 = np.tile(idx.astype(np.int16).reshape(-1, 16).T, (8, 1)).copy()
        dl = dl_all[seg].reshape(-1, 128).T.copy()
        ew = ew_all[seg].reshape(-1, 128).T.copy()
        per_core.append((idx16, dl, ew))
    # shared-program per-(local window, half) gather counts: max over cores
    maxA = cntA.reshape(NCORES, wpc).max(axis=0)
    maxB = cntB.reshape(NCORES, wpc).max(axis=0)
    return per_core, chA, chB, maxA, maxB


def _build_program(n_pad, wpc, chA, chB, maxA, maxB, b, in_c, out_c, k_hops, cols):
    rows = wpc * WIN
    ch = chA + chB
    nidx_w = ch * WIN
    totc = wpc * ch
    tot16 = wpc * nidx_w // 16
    koc = k_hops * out_c
    cmax = max(chA, chB)
    cseg = -(-cmax // -(-cmax // 9))  # balanced segments of <= 9 chunks

    nc = bacc.Bacc("TRN2", num_devices=NCORES, target_bir_lowering=False,
                   debug=False, num_swdge_queues=4,
                   dynamic_dma_scratch_size=40960)
    t_xT = nc.dram_tensor("xT", [in_c, b, rows], F32, kind="ExternalInput")
    t_hT = nc.dram_tensor("hT", [out_c, b, rows], F32, kind="ExternalInput")
    t_hrows = nc.dram_tensor("hrows", [rows, cols], F32, kind="ExternalInput")
    t_wx = nc.dram_tensor("wx", [in_c, koc], F32, kind="ExternalInput")
    t_wh = nc.dram_tensor("wh", [out_c, koc], F32, kind="ExternalInput")
    t_biasr = nc.dram_tensor("biasr", [128, cols], F32, kind="ExternalInput")
    t_iota = nc.dram_tensor("iota", [128, 128], F32, kind="ExternalInput")
    t_ident = nc.dram_tensor("ident", [128, 128], F32, kind="ExternalInput")
    t_idx = nc.dram_tensor("idx", [128, tot16], I16, kind="ExternalInput")
    t_dl = nc.dram_tensor("dstloc", [128, totc], F32, kind="ExternalInput")
    t_ew = nc.dram_tensor("ew", [128, totc], F32, kind="ExternalInput")
    t_out = nc.dram_tensor("out_rows", [rows, cols], F32, kind="ExternalOutput")

    with tile.TileContext(nc) as tc:
        with (tc.tile_pool(name="cons", bufs=1) as cons,
              tc.tile_pool(name="sbuf", bufs=3) as pool,
              tc.tile_pool(name="gbuf", bufs=6) as gbuf,
              tc.tile_pool(name="psum", bufs=3, space="PSUM") as psum,
              tc.tile_pool(name="psy", bufs=2, space="PSUM") as psy,
              tc.tile_pool(name="dram", bufs=3, space="DRAM") as dram):
            wx_t = cons.tile([in_c, koc], F32)
            nc.sync.dma_start(wx_t[:], t_wx[:])
            wh_t = cons.tile([out_c, koc], F32)
            nc.sync.dma_start(wh_t[:], t_wh[:])
            biasr_t = cons.tile([128, cols], F32)
            nc.sync.dma_start(biasr_t[:], t_biasr[:])
            iota_t = cons.tile([128, 128], F32)
            nc.sync.dma_start(iota_t[:], t_iota[:])
            ident_t = cons.tile([128, 128], F32)
            nc.sync.dma_start(ident_t[:], t_ident[:])
            idx_t = cons.tile([128, tot16], I16)
            nc.sync.dma_start(idx_t[:], t_idx[:])
            dl_t = cons.tile([128, totc], F32)
            nc.sync.dma_start(dl_t[:], t_dl[:])
            ew_t = cons.tile([128, totc], F32)
            nc.sync.dma_start(ew_t[:], t_ew[:])
            hbuf = cons.tile([128, wpc, cols], F32)
            nc.sync.dma_start(hbuf[:], t_hrows[:].rearrange("(w p) c -> p w c", p=128))
            zbuf = cons.tile([128, wpc, cols], F32)

            aw_r = SPLIT_W * WIN
            bw_r = rows - aw_r

            def ag_half(ag_h, tf_h):
                # AllGather of one half of each core's rows into its own
                # Shared half tensor (row = core*half_r + l % half_r).
                nc.gpsimd.collective_compute(
                    "AllGather", mybir.AluOpType.bypass,
                    replica_groups=[list(range(NCORES))],
                    ins=[ag_h.opt()], outs=[tf_h.opt()])

            def ag_slot(agA, agB, w):
                if w < SPLIT_W:
                    return agA, slice(w * WIN, (w + 1) * WIN)
                return agB, slice(w * WIN - aw_r, (w + 1) * WIN - aw_r)

            def y_window(second, w, agA, agB, ydram):
                # Y_k = H0 @ W_{k-1}; H0 = [x, h] or [x, z*h].  Writes Y_K
                # into ag_in and Y_{K-1}..Y_1 (f32) into ydram list.
                ystages = [pool.tile([128, cols], F32, tag=f"ys{j}",
                                      name=f"ys{j}")
                           for j in range(k_hops - 1)]
                ytop = pool.tile([128, cols], HDT, tag="ytop")
                for bi in range(b):
                    lx = pool.tile([in_c, 128], F32, tag="lx")
                    nc.sync.dma_start(
                        lx[:], t_xT[:, bi, w * WIN:(w + 1) * WIN])
                    lh = pool.tile([out_c, 128], F32, tag="lh")
                    if not second:
                        nc.sync.dma_start(
                            lh[:], t_hT[:, bi, w * WIN:(w + 1) * WIN])
                    else:
                        zh = pool.tile([128, out_c], F32, tag="zh")
                        csl = slice(bi * out_c, (bi + 1) * out_c)
                        nc.vector.tensor_mul(
                            zh[:], zbuf[:, w, csl], hbuf[:, w, csl])
                        tp = psy.tile([out_c, 128], F32, tag="tp")
                        nc.tensor.transpose(tp[:], zh[:], ident_t[:])
                        nc.vector.tensor_copy(lh[:], tp[:])
                    yp = psy.tile([128, koc], F32, tag="yp")
                    nc.tensor.matmul(yp[:], lhsT=lx[:], rhs=wx_t[:],
                                     start=True, stop=False)
                    nc.tensor.matmul(yp[:], lhsT=lh[:], rhs=wh_t[:],
                                     start=False, stop=True)
                    csl = slice(bi * out_c, (bi + 1) * out_c)
                    nc.vector.tensor_copy(
                        ytop[:, csl], yp[:, (k_hops - 1) * out_c:])
                    for j in range(k_hops - 1):
                        nc.vector.tensor_copy(
                            ystages[j][:, csl],
                            yp[:, j * out_c:(j + 1) * out_c])
                rsl = slice(w * WIN, (w + 1) * WIN)
                ag_t, asl = ag_slot(agA, agB, w)
                nc.sync.dma_start(ag_t[asl, :], ytop[:])
                for j in range(k_hops - 1):
                    nc.sync.dma_start(ydram[j][rsl, :], ystages[j][:])

            qctr = [0]

            def gather_half(tfh, w, c_lo, c_cnt, wcnt_max):
                # partial acc over one src-half's chunks of window w
                acc = psum.tile([128, cols], F32, tag="acc")
                for si, s0 in enumerate(range(0, c_cnt, cseg)):
                    ncs = min(cseg, c_cnt - s0)
                    c0 = c_lo + s0
                    # NOTE: num_idxs_reg < num_idxs hangs the runtime's sem
                    # accounting, so pads are gathered too (wcnt_max unused).
                    g = gbuf.tile([128, cseg, cols], HDT, tag="g")
                    off16 = w * (nidx_w // 16) + c0 * 8
                    qctr[0] += 1
                    nc.gpsimd.dma_gather(
                        out_ap=g[:, :ncs, :], in_ap=tfh[:],
                        idxs_ap=idx_t[:, off16:off16 + ncs * 8],
                        num_idxs=ncs * 128, num_idxs_reg=ncs * 128,
                        elem_size=cols, single_packet=False,
                        queue_num=qctr[0] % 4)
                    s_big = gbuf.tile([128, cseg * 128], HDT, tag="sbig")
                    s3 = s_big[:, :ncs * 128].rearrange(
                        "p (c j) -> p c j", c=ncs)
                    csl = slice(w * ch + c0, w * ch + c0 + ncs)
                    dl_exp = dl_t[:, csl] \
                        .rearrange("p (c o) -> p c o", o=1) \
                        .to_broadcast([128, ncs, 128])
                    ew_exp = ew_t[:, csl] \
                        .rearrange("p (c o) -> p c o", o=1) \
                        .to_broadcast([128, ncs, 128])
                    iota_exp = iota_t[:].rearrange("p (c j) -> p c j", c=1) \
                        .to_broadcast([128, ncs, 128])
                    nc.vector.tensor_tensor(out=s3, in0=iota_exp, in1=dl_exp,
                                            op=mybir.AluOpType.is_equal)
                    nc.vector.tensor_tensor(out=s3, in0=s3, in1=ew_exp,
                                            op=mybir.AluOpType.mult)
                    for ci in range(ncs):
                        nc.tensor.matmul(
                            acc[:], lhsT=s_big[:, ci * 128:(ci + 1) * 128],
                            rhs=g[:, ci, :], start=(s0 + ci == 0),
                            stop=(s0 + ci == c_cnt - 1))
                return acc

            def a_window(tfA, ydram_j, part, w, is_final):
                # pass 1: A-chunk partial + Y_j (or bias) -> part[w] in DRAM
                acc = gather_half(tfA, w, 0, chA, maxA[w])
                pw = pool.tile([128, cols], F32, tag="pw")
                if is_final:
                    nc.vector.tensor_add(pw[:], acc[:], biasr_t[:])
                else:
                    yj = pool.tile([128, cols], F32, tag="yj")
                    nc.sync.dma_start(yj[:], ydram_j[w * WIN:(w + 1) * WIN, :])
                    nc.vector.tensor_add(pw[:], acc[:], yj[:])
                nc.sync.dma_start(part[w * WIN:(w + 1) * WIN, :], pw[:])

            def mid_window_b(tfB, part, agA_n, agB_n, w):
                # pass 2: B-chunk partial + staged partial -> next hop input
                acc = gather_half(tfB, w, chA, chB, maxB[w])
                pl = pool.tile([128, cols], F32, tag="pl")
                nc.sync.dma_start(pl[:], part[w * WIN:(w + 1) * WIN, :])
                urow = pool.tile([128, cols], HDT, tag="urow")
                nc.vector.tensor_add(urow[:], acc[:], pl[:])
                ag_t, asl = ag_slot(agA_n, agB_n, w)
                nc.sync.dma_start(ag_t[asl, :], urow[:])

            def final_window_b(tfB, part, w, second):
                acc = gather_half(tfB, w, chA, chB, maxB[w])
                pl = pool.tile([128, cols], F32, tag="pl")
                nc.sync.dma_start(pl[:], part[w * WIN:(w + 1) * WIN, :])
                tmpf = pool.tile([128, cols], F32, tag="tmpf")
                nc.vector.tensor_add(tmpf[:], acc[:], pl[:])
                if not second:
                    nc.scalar.activation(
                        zbuf[:, w, :], tmpf[:],
                        mybir.ActivationFunctionType.Sigmoid)
                else:
                    htl = pool.tile([128, cols], F32, tag="htl")
                    nc.scalar.activation(
                        htl[:], tmpf[:], mybir.ActivationFunctionType.Tanh)
                    d = pool.tile([128, cols], F32, tag="d")
                    nc.vector.tensor_sub(d[:], hbuf[:, w, :], htl[:])
                    nc.vector.tensor_mul(d[:], zbuf[:, w, :], d[:])
                    outw = pool.tile([128, cols], F32, tag="outw")
                    nc.vector.tensor_add(outw[:], d[:], htl[:])
                    nc.sync.dma_start(
                        t_out[w * WIN:(w + 1) * WIN, :], outw[:])

            def new_chain():
                agA = dram.tile([aw_r, cols], HDT, tag="agA")
                agB = dram.tile([bw_r, cols], HDT, tag="agB")
                tfA = dram.tile([NCORES * aw_r, cols], HDT, tag="tfA",
                                addr_space="Shared")
                tfB = dram.tile([NCORES * bw_r, cols], HDT, tag="tfB",
                                addr_space="Shared")
                return agA, agB, tfA, tfB

            part_ctr = [0]

            def new_part():
                part_ctr[0] += 1
                return dram.tile([rows, cols], F32, tag="part",
                                 name=f"part{part_ctr[0]}")

            # AG for part A of the next-hop input fires a couple of windows
            # after its inputs are stored (in-order gpsimd queue).
            FIRE0 = SPLIT_W + 1

            # ---- dconv 1: Y stage ----
            ydram = [dram.tile([rows, cols], F32, tag=f"yd{j}", name=f"yd{j}")
                     for j in range(k_hops - 1)]
            agA, agB, tfA, tfB = new_chain()
            for w in range(wpc):
                y_window(False, w, agA, agB, ydram)
                if w == FIRE0:
                    ag_half(agA, tfA)
            ag_half(agB, tfB)

            def hop_phases(ydrams, second):
                nonlocal_tf = {}

                def run(tfA, tfB):
                    tfs = (tfA, tfB)
                    for j in range(k_hops - 2, -1, -1):
                        part = new_part()
                        for w in range(wpc):
                            a_window(tfs[0], ydrams[j], part, w, False)
                        agA_n, agB_n, tfA_n, tfB_n = new_chain()
                        for w in range(wpc):
                            mid_window_b(tfs[1], part, agA_n, agB_n, w)
                            if w == FIRE0:
                                ag_half(agA_n, tfA_n)
                        ag_half(agB_n, tfB_n)
                        tfs = (tfA_n, tfB_n)
                    return tfs

                return run

            # ---- dconv 1: mid hops ----
            tfA, tfB = hop_phases(ydram, False)(tfA, tfB)
            # ---- final hop of dconv 1 merged with y_stage of dconv 2 ----
            ydram2 = [dram.tile([rows, cols], F32, tag=f"yd{j}", name=f"y2d{j}")
                      for j in range(k_hops - 1)]
            part = new_part()
            for w in range(wpc):
                a_window(tfA, None, part, w, True)
            agA2, agB2, tfA2, tfB2 = new_chain()
            for w in range(wpc):
                final_window_b(tfB, part, w, second=False)
                y_window(True, w, agA2, agB2, ydram2)
                if w == FIRE0:
                    ag_half(agA2, tfA2)
            ag_half(agB2, tfB2)
            # ---- dconv 2: mid hops ----
            tfA, tfB = hop_phases(ydram2, True)(tfA2, tfB2)
            # ---- final hop of dconv 2 ----
            part = new_part()
            for w in range(wpc):
                a_window(tfA, None, part, w, True)
            for w in range(wpc):
                final_window_b(tfB, part, w, second=True)

    nc.compile()
    return nc


def kernel(x, h, weight, bias, edge_index, edge_weight):
    x = np.asarray(x, np.float32)
    h = np.asarray(h, np.float32)
    weight = np.asarray(weight, np.float32)
    bias = np.asarray(bias, np.float32)
    edge_index = np.asarray(edge_index)
    edge_weight = np.asarray(edge_weight, np.float32)

    b, n, in_c = x.shape
    out_c = h.shape[2]
    k_hops = weight.shape[0]
    cols = b * out_c
    assert (cols * 4) % 256 == 0, cols

    n_pad = -(-n // (WIN * NCORES)) * WIN * NCORES
    wpc = n_pad // WIN // NCORES
    rows = wpc * WIN

    per_core_edges, chA, chB, maxA, maxB = _prep_edges(edge_index, edge_weight, n_pad, wpc)
    nc = _build_program(n_pad, wpc, chA, chB, maxA, maxB, b, in_c, out_c, k_hops, cols)

    x_pad = np.zeros((b, n_pad, in_c), np.float32)
    x_pad[:, :n] = x
    h_pad = np.zeros((b, n_pad, out_c), np.float32)
    h_pad[:, :n] = h
    xT = np.ascontiguousarray(x_pad.transpose(2, 0, 1))       # [in_c, b, n_pad]
    hT = np.ascontiguousarray(h_pad.transpose(2, 0, 1))       # [out_c, b, n_pad]
    hrows = np.ascontiguousarray(
        h_pad.transpose(1, 0, 2).reshape(n_pad, cols))        # [n_pad, b*out_c]
    w_all = np.ascontiguousarray(
        weight.transpose(1, 0, 2).reshape(in_c + out_c, k_hops * out_c))
    biasr = np.tile(bias, (128, b)).astype(np.float32)
    iota = np.broadcast_to(np.arange(128, dtype=np.float32), (128, 128)).copy()
    ident = np.eye(128, dtype=np.float32)

    in_maps = []
    for c in range(NCORES):
        rsl = slice(c * rows, (c + 1) * rows)
        idx16, dl, ew = per_core_edges[c]
        in_maps.append({
            "xT": np.ascontiguousarray(xT[:, :, rsl]),
            "hT": np.ascontiguousarray(hT[:, :, rsl]),
            "hrows": np.ascontiguousarray(hrows[rsl]),
            "wx": np.ascontiguousarray(w_all[:in_c]),
            "wh": np.ascontiguousarray(w_all[in_c:]),
            "biasr": biasr, "iota": iota, "ident": ident,
            "idx": idx16, "dstloc": dl, "ew": ew,
        })

    res = bass_utils.run_bass_kernel_spmd(nc, in_maps, core_ids=list(range(NCORES)))
    kernel._last_results = res

    full = np.concatenate([res.results[c]["out_rows"] for c in range(NCORES)], axis=0)
    out = full[:n].reshape(n, b, out_c).transpose(1, 0, 2)
    return np.ascontiguousarray(out, dtype=np.float32)

